# revision 16
# baseline (speedup 1.0000x reference)
import sys
if "/opt/trn_rl_repo" not in sys.path:
    sys.path.insert(0, "/opt/trn_rl_repo")

import numpy as np
import ml_dtypes
import concourse.bacc as bacc
import concourse.tile as tile
from concourse import mybir
from concourse.bass_utils import run_bass_kernel_spmd

B, S, D = 4, 2048, 1024
NCORES = 8
F32 = mybir.dt.float32
F32R = mybir.dt.float32r
BF16 = mybir.dt.bfloat16
_cache = {}


def _build(reps=1):
    if reps in _cache:
        return _cache[reps]
    nc = bacc.Bacc()
    xt = nc.dram_tensor("xt", [D, B * S], BF16, kind="ExternalInput")
    wq = nc.dram_tensor("wq", [128, D], BF16, kind="ExternalInput")
    wk = nc.dram_tensor("wk", [128, D], BF16, kind="ExternalInput")
    wv = nc.dram_tensor("wv", [128, D], BF16, kind="ExternalInput")
    wo = nc.dram_tensor("wo", [128, D], F32R, kind="ExternalInput")
    bq = nc.dram_tensor("bq", [128, 1], F32, kind="ExternalInput")
    bk = nc.dram_tensor("bk", [128, 1], F32, kind="ExternalInput")
    idm = nc.dram_tensor("idm", [128, 128], F32R, kind="ExternalInput")
    on64 = nc.dram_tensor("on64", [128, 64], F32R, kind="ExternalInput")
    po = nc.dram_tensor("po", [B * D, S], F32, kind="ExternalOutput")
    warm = nc.dram_tensor("warm", [128, 512], F32, kind="ExternalOutput")

    ACT = mybir.ActivationFunctionType

    with tile.TileContext(nc) as tc:
        with tc.tile_pool(name="sb", bufs=1) as sb, \
             tc.tile_pool(name="ps", bufs=2, space="PSUM") as ps:
            wq_sb = sb.tile([128, D], BF16)
            wk_sb = sb.tile([128, D], BF16)
            wv_sb = sb.tile([128, D], BF16)
            wo_sb = sb.tile([128, D], F32R)
            bq_sb = sb.tile([128, 1], F32)
            bk_sb = sb.tile([128, 1], F32)
            nc.sync.dma_start(out=wq_sb, in_=wq[:, :])
            nc.sync.dma_start(out=wk_sb, in_=wk[:, :])
            nc.sync.dma_start(out=wv_sb, in_=wv[:, :])
            nc.sync.dma_start(out=wo_sb, in_=wo[:, :])
            nc.sync.dma_start(out=bq_sb, in_=bq[:, :])
            nc.sync.dma_start(out=bk_sb, in_=bk[:, :])

            ident = sb.tile([128, 128], F32R)
            nc.sync.dma_start(out=ident, in_=idm[:, :])
            # vp: 16 sk-tiles x (64 ones | 64 V_h0 | 64 ones | 64 V_h1) =
            # 256 cols.  PV lhsT for head h = cols [h*128:(h+1)*128] =
            # [1 | V_h]: the ones block rides along in the matmul and lands
            # the softmax denominator on PSUM rows 0:64 (a free broadcast),
            # ctx on rows 64:128 -- uniform for both heads, and the
            # reciprocal stays at base partition 0 (required: the custom
            # DVE op miscomputes at nonzero base partitions).
            vp = sb.tile([128, 16 * 256], F32R)
            for t in range(16):
                nc.sync.dma_start(
                    out=vp[:, t * 256:t * 256 + 64], in_=on64[:, :])
                nc.sync.dma_start(
                    out=vp[:, t * 256 + 128:t * 256 + 192], in_=on64[:, :])

            qt = [sb.tile([128, S], F32R, name=f"qt{i}") for i in range(2)]
            kt = [sb.tile([128, S], F32R, name=f"kt{i}") for i in range(2)]
            vt = [sb.tile([128, S], F32R, name=f"vt{i}") for i in range(2)]
            ctxT = sb.tile([128, S], F32R)

            def emit_xs(bi):
                xsl = []
                for k in range(8):
                    xs = sb.tile([128, S], BF16, tag="xs", bufs=8)
                    nc.sync.dma_start(
                        out=xs,
                        in_=xt[k * 128:(k + 1) * 128, bi * S:(bi + 1) * S])
                    xsl.append(xs)
                return xsl

            wbt = ((wq_sb, bq_sb), (wk_sb, bk_sb), (wv_sb, None))

            def emit_qkv_group(g, xsl, par):
                proj, half = g // 2, g % 2
                wt, bt = wbt[proj]
                dst = (qt, kt, vt)[proj][par]
                pq = ps.tile([128, 1024], F32, tag="aux", bufs=1)
                for n2 in range(2):
                    c0 = half * 1024 + n2 * 512
                    for k in range(8):
                        nc.tensor.matmul(
                            pq[:, n2 * 512:(n2 + 1) * 512],
                            wt[:, k * 128:(k + 1) * 128],
                            xsl[k][:, c0:c0 + 512],
                            start=(k == 0), stop=(k == 7))
                # flush on DVE (ACT is the exp-bound engine)
                dsl = dst[:, half * 1024:(half + 1) * 1024]
                if bt is None:
                    nc.vector.tensor_copy(out=dsl, in_=pq[:, :])
                else:
                    nc.vector.tensor_scalar_add(
                        out=dsl, in0=pq[:, :], scalar1=bt[:, 0:1])

            seq = list(range(B)) * reps
            # prologue: load + project batch seq[0] into parity 0.  The
            # first x tiles take ~12us to DMA in; run dummy accumulating
            # matmuls meanwhile so the PE p-state ramp is warm (2.4 GHz)
            # by the time real work arrives.
            wp = ps.tile([128, 512], F32, tag="aux", bufs=1)
            for w in range(28):
                nc.tensor.matmul(wp, ident[:, :], wo_sb[:, 0:512],
                                 start=(w == 0), stop=(w == 27))
            wfl = sb.tile([128, 512], F32)
            nc.vector.tensor_copy(out=wfl, in_=wp[:, :])
            nc.sync.dma_start(out=warm[:, :], in_=wfl)
            xsl = emit_xs(seq[0])
            for g in range(6):
                emit_qkv_group(g, xsl, 0)

            def emit_vprime(t, par, tag):
                tp = ps.tile([128, 1024], F32R, tag=tag,
                             bufs=(2 if tag == "pa" else 1))
                nc.tensor.transpose(
                    tp[:, 0:128], vt[par][:, t * 128:(t + 1) * 128],
                    ident[:, :])
                nc.vector.tensor_copy(
                    out=vp[:, t * 256 + 64:t * 256 + 128],
                    in_=tp[:, 0:64])
                nc.vector.tensor_copy(
                    out=vp[:, t * 256 + 192:t * 256 + 256],
                    in_=tp[:, 64:128])

            first = True
            for i, b in enumerate(seq):
                par = i % 2
                nxt = seq[(i + 1) % len(seq)]
                xsl = emit_xs(nxt)

                if first:
                    # no stage-loop slack to hide it in on the first batch
                    for t in range(16):
                        emit_vprime(t, par, "pa")
                    first = False

                # attention: flattened (h,j,t) pipeline, PV lags scores by
                # TWO stages so the exp sem is satisfied before PV reaches
                # the head of the in-order PE queue (any wait resets the
                # p-state ramp).  QKV bursts for the next batch land right
                # after each PV accumulation finishes, covering the
                # (reciprocal, multiply) tail on DVE; next batch's V'
                # transposes hide in the late-stage slack on the "aux"
                # PSUM slot, which the qkv bursts have vacated by then.
                ets = {}
                cxps = {}
                if i + 1 < len(seq):
                    qkv_sched = {18: 0, 19: 1, 34: 2, 35: 3, 50: 4, 51: 5}
                else:
                    # last batch: only the bursts that cover the p0/p1/p2
                    # softmax tails are worth emitting
                    qkv_sched = {18: 0, 34: 1, 50: 2}

                def emit_outproj(g16):
                    half, m = g16 // 8, g16 % 8
                    tag = "aux" if half == 0 else "pa"
                    pso = ps.tile([128, 1024], F32, tag=tag,
                                  bufs=(1 if half == 0 else 2))
                    for n2 in range(2):
                        c0 = half * 1024 + n2 * 512
                        nc.tensor.matmul(
                            pso[:, n2 * 512:(n2 + 1) * 512],
                            wo_sb[:, m * 128:(m + 1) * 128],
                            ctxT[:, c0:c0 + 512], start=True, stop=True)
                    ob = sb.tile([128, 1024], F32, tag="ob", bufs=4)
                    if g16 % 2 == 0:
                        nc.vector.tensor_copy(out=ob, in_=pso[:, :])
                    else:
                        nc.scalar.copy(out=ob, in_=pso[:, :])
                    nc.sync.dma_start(
                        out=po[b * D + m * 128:b * D + (m + 1) * 128,
                               half * 1024:(half + 1) * 1024],
                        in_=ob)

                for s in range(66):
                    if s < 64:
                        p, t = s // 16, s % 16
                        h, j = p // 2, p % 2
                        scp = ps.tile([128, 1024], F32, tag="pa", bufs=2)
                        for n2 in range(2):
                            q0 = j * 1024 + n2 * 512
                            nc.tensor.matmul(
                                scp[:, n2 * 512:(n2 + 1) * 512],
                                kt[par][h * 64:(h + 1) * 64,
                                        t * 128:(t + 1) * 128],
                                qt[par][h * 64:(h + 1) * 64, q0:q0 + 512],
                                start=True, stop=True)
                        et = sb.tile([128, 1024], F32R, tag="et", bufs=4)
                        nc.scalar.activation(
                            out=et, in_=scp, func=ACT.Exp, scale=0.125)
                        ets[s] = et
                    if s in qkv_sched:
                        emit_qkv_group(qkv_sched[s], xsl, 1 - par)
                    if s >= 2:
                        p1, t1 = (s - 2) // 16, (s - 2) % 16
                        h1 = p1 // 2
                        if t1 == 0:
                            cxp_new = ps.tile([128, 1024], F32, tag="cx",
                                              bufs=1)
                            cxps[p1] = cxp_new
                        et1 = ets.pop(s - 2)
                        for n2 in range(2):
                            nc.tensor.matmul(
                                cxps[p1][:, n2 * 512:(n2 + 1) * 512],
                                vp[:, t1 * 256 + h1 * 128:
                                   t1 * 256 + (h1 + 1) * 128],
                                et1[:, n2 * 512:(n2 + 1) * 512],
                                start=(t1 == 0), stop=(t1 == 15))
                        if t1 == 15:
                            j1 = p1 % 2
                            cxp = cxps.pop(p1)
                            # Z on rows 0:64, ctx on rows 64:128
                            bcs = sb.tile([64, 1024], F32, tag="bcs",
                                          bufs=2)
                            nc.vector.reciprocal_approx_fast(
                                out=bcs, in_=cxp[0:64, :])
                            nc.vector.tensor_tensor(
                                ctxT[h1 * 64:(h1 + 1) * 64,
                                     j1 * 1024:(j1 + 1) * 1024],
                                cxp[64:128, :], bcs[:, :],
                                mybir.AluOpType.mult)
                    # out-proj half=0 (j0 queries) spread through the late
                    # stages: its ctxT columns are complete once the p2
                    # tail lands (~stage 51), and the qkv bursts have
                    # vacated the "aux" PSUM slot by then.
                    if 52 <= s <= 59:
                        emit_outproj(s - 52)

                # batch end: out-proj half=1 interleaved with the next
                # batch's V' transposes.  The first vprimes also cover
                # p3's reciprocal/multiply tail, which half=1 depends on.
                for m8 in range(8):
                    if i + 1 < len(seq):
                        emit_vprime(2 * m8, (i + 1) % 2, "aux")
                        emit_vprime(2 * m8 + 1, (i + 1) % 2, "aux")
                    emit_outproj(8 + m8)
    nc.finalize()
    _cache[reps] = nc
    return nc


def _warr(W):
    # W [128 outdims, 1024 indims] -> SBUF lhsT layout [128 p, 8k x 128 m]
    return np.ascontiguousarray(
        W.reshape(128, 8, 128).transpose(2, 1, 0).reshape(128, 1024))


def _in_maps(x, qkv_w, qkv_b, out_w):
    xT = np.ascontiguousarray(
        x.reshape(B * S, D).T).astype(ml_dtypes.bfloat16)
    in_maps = []
    for c in range(NCORES):
        base = c * 128
        V = out_w[:, base:base + 128]
        in_maps.append({
            "xt": xT,
            "wq": _warr(qkv_w[base:base + 128, :]).astype(ml_dtypes.bfloat16),
            "wk": _warr(qkv_w[D + base:D + base + 128, :]).astype(ml_dtypes.bfloat16),
            "wv": _warr(qkv_w[2 * D + base:2 * D + base + 128, :]).astype(ml_dtypes.bfloat16),
            "wo": np.ascontiguousarray(
                V.reshape(8, 128, 128).transpose(2, 0, 1).reshape(128, 1024)
            ).astype(np.float32),
            "idm": np.eye(128, dtype=np.float32),
            "on64": np.ones((128, 64), dtype=np.float32),
            "bq": qkv_b[base:base + 128].reshape(128, 1).astype(np.float32),
            "bk": qkv_b[D + base:D + base + 128].reshape(128, 1).astype(np.float32),
        })
    return in_maps


def kernel(x, qkv_w, qkv_b, out_w, out_b):
    nc = _build()
    in_maps = _in_maps(x, qkv_w, qkv_b, out_w)
    res = run_bass_kernel_spmd(nc, in_maps, core_ids=list(range(NCORES)),
                               trace=False)
    kernel.last_exec_ns = res.exec_time_ns
    acc = np.zeros((B, D, S), dtype=np.float64)
    for c in range(NCORES):
        acc += res.results[c]["po"].reshape(B, D, S)
    # v-bias folds into the output bias: ctx = attn@(v+bv) = attn@v + bv
    # (attention rows sum to 1), so out += out_w @ bv is exact.
    out_b_eff = out_b.astype(np.float64) + \
        out_w.astype(np.float64) @ qkv_b[2 * D:3 * D].astype(np.float64)
    out = acc.transpose(0, 2, 1) + out_b_eff
    return out.astype(np.float32)
